# revision 4
# baseline (speedup 1.0000x reference)
"""Trainium2 Bass kernel for nn_CrossAttention_Sp_10909216932413.

Strategy: sequence-parallel over the query/patch axis N=1024 across 8 cores
(128 queries per core, all B=8 batches). Softmax/entropy reduce only over the
key axis, so no cross-core communication is needed. Each core reads its x/coords
query slice, the full y, and the (replicated) small params.

Per-core pipeline (all fused on chip, fp32):
  pos:    pos_logits = sum_c coords*pos_emb -> exp(-|p_temp| * .) -> posg = g*softmax
  batch:  xT (PE transpose), tT = U^T-chunks.T @ xT, ts_k = tT * |S_k|,
          xs_kT = U-chunks.T @ ts_k, yT (PE transpose),
          scores_k = xs_kT.T @ yT (PSUM), e_k = Exp(scale*scores) + row sum (ACT),
          attn_k = e_k * ((1-g)/sum) + posg, S_k = sum(attn*ln(attn+eps)),
          route: sg_k = sigmoid(h_temp*S_k); heat = 2*max(sg); mask = sg0>=sg1,
          attn_sel = copy_predicated(attn1, mask, attn0),
          out = attn_selT.T @ y_nat  (PE transpose + matmul accumulation).
"""

import numpy as np

from concourse import bacc, mybir, tile
from concourse.bass_utils import run_bass_kernel_spmd
from concourse.masks import make_identity

F32 = mybir.dt.float32
U8 = mybir.dt.uint8
AF = mybir.ActivationFunctionType
ALU = mybir.AluOpType

B, N, D = 8, 1024, 256
P = 128                # partitions / queries per core
NCORES = 8
EPS = 1e-8
SCALE = D ** -0.5

_CACHE = {}


def _build(g: float, h_temp: float, p_temp: float, reps: int = 1):
    nc = bacc.Bacc("TRN2", target_bir_lowering=False, debug=False,
                   enable_asserts=True, num_devices=NCORES)

    x_d = nc.dram_tensor("x", [B, P, D], F32, kind="ExternalInput").ap()
    y_d = nc.dram_tensor("y", [B, N, D], F32, kind="ExternalInput").ap()
    coords_d = nc.dram_tensor("coords", [P, N, 6], F32, kind="ExternalInput").ap()
    pe_d = nc.dram_tensor("pe", [P, 6], F32, kind="ExternalInput").ap()
    u_d = nc.dram_tensor("u", [D, D], F32, kind="ExternalInput").ap()
    s_d = nc.dram_tensor("s", [D, 2], F32, kind="ExternalInput").ap()
    out_d = nc.dram_tensor("out", [B, P, D], F32, kind="ExternalOutput").ap()
    heat_d = nc.dram_tensor("heat", [B, P, 1], F32, kind="ExternalOutput").ap()

    with tile.TileContext(nc) as tc:
        with (
            tc.tile_pool(name="const", bufs=1) as cpool,
            tc.tile_pool(name="pos", bufs=1) as ppool,
            tc.tile_pool(name="sb", bufs=2) as sb,
            tc.tile_pool(name="big", bufs=2) as big,
            tc.tile_pool(name="ps_s", bufs=2, space="PSUM") as ps_s,
            tc.tile_pool(name="ps_w", bufs=3, space="PSUM") as ps_w,
            tc.tile_pool(name="ps_o", bufs=1, space="PSUM") as ps_o,
        ):
            # ---- constants ----
            ident = cpool.tile([P, P], F32)
            make_identity(nc, ident[:])
            eps_t = cpool.tile([P, 1], F32)
            nc.vector.memset(eps_t[:], EPS)

            # U natural: 2 chunks of rows d' -> [128, 2(d'c), 256(e)]
            u_sb = cpool.tile([P, 2, D], F32)
            nc.sync.dma_start(out=u_sb[:], in_=u_d.rearrange("(c p) e -> p c e", p=P))
            # U^T: [128, 2(dc), 256(d')] via 4 PE transposes
            ut_sb = cpool.tile([P, 2, D], F32)
            for dc in range(2):
                tr = ps_w.tile([P, 2 * P], F32, tag="ps_w")
                for dpc in range(2):
                    nc.tensor.transpose(
                        tr[:, dpc * P:(dpc + 1) * P],
                        u_sb[:, dpc, dc * P:(dc + 1) * P], ident[:])
                nc.scalar.copy(ut_sb[:, dc, :], tr[:])

            # |S1|, |S2| as per-(d'-chunk) per-partition scalars: [128, 2(d'c), 2(k)]
            s_sb = cpool.tile([P, 2, 2], F32)
            nc.sync.dma_start(out=s_sb[:], in_=s_d.rearrange("(c p) k -> p c k", p=P))
            s_abs = cpool.tile([P, 2, 2], F32)
            nc.scalar.activation(s_abs[:], s_sb[:], AF.Abs)

            # ---- repeat wrapper (timing only; reps=1 -> no loop) ----
            import contextlib
            rep_ctx = tc.For_i(0, reps, 1) if reps > 1 else contextlib.nullcontext()
            with rep_ctx:
                _body(nc, tc, g, h_temp, p_temp,
                      cpool, ppool, sb, big, ps_s, ps_w, ps_o,
                      ident, eps_t, u_sb, ut_sb, s_abs,
                      x_d, y_d, coords_d, pe_d, out_d, heat_d)

    nc.compile()
    return nc


def _body(nc, tc, g, h_temp, p_temp,
          cpool, ppool, sb, big, ps_s, ps_w, ps_o,
          ident, eps_t, u_sb, ut_sb, s_abs,
          x_d, y_d, coords_d, pe_d, out_d, heat_d):
    if True:
        if True:
            # ---- positional scores (once per core) ----
            coords_t = ppool.tile([P, N, 6], F32)
            nc.sync.dma_start(out=coords_t[:], in_=coords_d[:])
            pe_t = ppool.tile([P, 1, 6], F32)
            nc.sync.dma_start(out=pe_t[:, 0, :], in_=pe_d[:])
            prod = ppool.tile([P, N, 6], F32)
            nc.vector.tensor_tensor(prod[:], coords_t[:],
                                    pe_t[:, 0:1, :].broadcast_to([P, N, 6]), ALU.mult)
            pl = ppool.tile([P, N], F32)
            nc.vector.tensor_reduce(pl[:], prod[:], axis=mybir.AxisListType.X,
                                    op=ALU.add)
            pos_e = ppool.tile([P, N], F32)
            pos_s = ppool.tile([P, 1], F32)
            nc.scalar.activation(pos_e[:], pl[:], AF.Exp, scale=-abs(p_temp),
                                 accum_out=pos_s[:])
            pos_r = ppool.tile([P, 1], F32)
            nc.vector.reciprocal(pos_r[:], pos_s[:])
            pos_rg = ppool.tile([P, 1], F32)
            nc.vector.tensor_scalar_mul(pos_rg[:], pos_r[:], g)
            posg = ppool.tile([P, N], F32)
            nc.vector.tensor_scalar_mul(posg[:], pos_e[:], pos_rg[:])

            # ---- per-batch pipeline ----
            for b in range(B):
                # x slice + transpose -> xT [128, 2(dc), 128(q)]
                x_t = sb.tile([P, D], F32, tag="x")
                nc.sync.dma_start(out=x_t[:], in_=x_d[b])
                xT = sb.tile([P, 2 * P], F32, tag="xT")
                trx = ps_w.tile([P, 2 * P], F32, tag="ps_w")
                for dc in range(2):
                    nc.tensor.transpose(trx[:, dc * P:(dc + 1) * P],
                                        x_t[:, dc * P:(dc + 1) * P], ident[:])
                nc.vector.tensor_copy(xT[:], trx[:])

                # tT[d', q] chunks: psum [128, 2(d'c), 128]
                tT = ps_w.tile([P, 2 * P], F32, tag="ps_w")
                for dpc in range(2):
                    for dc in range(2):
                        nc.tensor.matmul(
                            tT[:, dpc * P:(dpc + 1) * P],
                            ut_sb[:, dc, dpc * P:(dpc + 1) * P],
                            xT[:, dc * P:(dc + 1) * P],
                            start=(dc == 0), stop=(dc == 1))
                # ts_k = tT * |S_k| (read PSUM, write SBUF); col (k*2+dpc)*P
                tsT = sb.tile([P, 4 * P], F32, tag="tsT")
                for k in range(2):
                    for dpc in range(2):
                        nc.vector.tensor_scalar_mul(
                            tsT[:, (k * 2 + dpc) * P:(k * 2 + dpc + 1) * P],
                            tT[:, dpc * P:(dpc + 1) * P],
                            s_abs[:, dpc, k:k + 1])

                # xs_kT[e, q]: psum [128, 2(ec), 128] per k -> sbuf
                xsT = sb.tile([P, 4 * P], F32, tag="xsT")  # col (k*2+ec)*P
                for k in range(2):
                    xs_ps = ps_w.tile([P, 2 * P], F32, tag="ps_w")
                    for ec in range(2):
                        for dpc in range(2):
                            nc.tensor.matmul(
                                xs_ps[:, ec * P:(ec + 1) * P],
                                u_sb[:, dpc, ec * P:(ec + 1) * P],
                                tsT[:, (k * 2 + dpc) * P:(k * 2 + dpc + 1) * P],
                                start=(dpc == 0), stop=(dpc == 1))
                    nc.scalar.copy(xsT[:, k * 2 * P:(k + 1) * 2 * P], xs_ps[:])

                # y[b] natural [128, 8(mc), 256(e)]
                y_nat = big.tile([P, 8, D], F32, tag="y_nat")
                nc.sync.dma_start(out=y_nat[:],
                                  in_=y_d[b].rearrange("(mc p) e -> p mc e", p=P))
                # yT [128, 2(ec), 1024(m)] via 16 PE transposes
                yT = big.tile([P, 2 * N], F32, tag="yT")  # col ec*N + m
                for ec in range(2):
                    for mg in range(2):
                        tr = ps_w.tile([P, 4 * P], F32, tag="ps_w")
                        for i in range(4):
                            mc = mg * 4 + i
                            nc.tensor.transpose(
                                tr[:, i * P:(i + 1) * P],
                                y_nat[:, mc, ec * P:(ec + 1) * P], ident[:])
                        dst = yT[:, ec * N + mg * 512:ec * N + (mg + 1) * 512]
                        if (ec + mg) % 2 == 0:
                            nc.scalar.copy(dst, tr[:])
                        else:
                            nc.vector.tensor_copy(dst, tr[:])

                # scores + softmax + entropy per branch
                e_t = [None, None]
                attn = [None, None]
                c_t = [None, None]
                ent2 = sb.tile([P, 2], F32, tag="ent")
                for k in range(2):
                    sc_ps = ps_s.tile([P, N], F32, tag="scores")
                    for mh in range(2):
                        for ec in range(2):
                            nc.tensor.matmul(
                                sc_ps[:, mh * 512:(mh + 1) * 512],
                                xsT[:, (k * 2 + ec) * P:(k * 2 + ec + 1) * P],
                                yT[:, ec * N + mh * 512:ec * N + (mh + 1) * 512],
                                start=(ec == 0), stop=(ec == 1))
                    e_t[k] = big.tile([P, N], F32, tag=f"e{k}", name=f"e{k}_{b}")
                    s_e = sb.tile([P, 1], F32, tag=f"se{k}")
                    nc.scalar.activation(e_t[k][:], sc_ps[:], AF.Exp, scale=SCALE,
                                         accum_out=s_e[:])
                    r_t = sb.tile([P, 1], F32, tag=f"r{k}")
                    nc.vector.reciprocal(r_t[:], s_e[:])
                    c_t[k] = sb.tile([P, 1], F32, tag=f"c{k}", name=f"c{k}_{b}")
                    nc.vector.tensor_scalar_mul(c_t[k][:], r_t[:], 1.0 - g)
                    # attn_k = e_k * c_k + posg
                    tk = big.tile([P, N], F32, tag=f"t{k}")
                    nc.vector.tensor_scalar_mul(tk[:], e_t[k][:], c_t[k][:])
                    attn[k] = big.tile([P, N], F32, tag=f"attn{k}", name=f"attn{k}_{b}")
                    nc.gpsimd.tensor_tensor(attn[k][:], tk[:], posg[:], ALU.add)
                    # S_k = sum attn*ln(attn+eps)
                    lg = big.tile([P, N], F32, tag="lg")
                    nc.scalar.activation(lg[:], attn[k][:], AF.Ln, bias=eps_t[:])
                    el = big.tile([P, N], F32, tag="el")
                    nc.gpsimd.tensor_tensor(el[:], attn[k][:], lg[:], ALU.mult)
                    nc.vector.tensor_reduce(ent2[:, k:k + 1], el[:],
                                            axis=mybir.AxisListType.X, op=ALU.add)

                # routing
                sg = sb.tile([P, 2], F32, tag="sg")
                nc.scalar.activation(sg[:], ent2[:], AF.Sigmoid, scale=h_temp)
                heat_t = sb.tile([P, 1], F32, tag="heat")
                nc.vector.tensor_tensor(heat_t[:], sg[:, 0:1], sg[:, 1:2], ALU.max)
                nc.vector.tensor_scalar_mul(heat_t[:], heat_t[:], 2.0)
                nc.sync.dma_start(out=heat_d[b], in_=heat_t[:])
                mask = sb.tile([P, 1], U8, tag="mask")
                nc.vector.tensor_tensor(mask[:], sg[:, 0:1], sg[:, 1:2], ALU.is_ge)
                # attn_sel = attn1 overwritten by attn0 where mask
                nc.vector.copy_predicated(attn[1][:], mask[:].broadcast_to([P, N]),
                                          attn[0][:])

                # A^T [128, 8(mc), 128(q)]
                aT = sb.tile([P, 8 * P], F32, tag="aT")
                for mg in range(2):
                    tr = ps_w.tile([P, 4 * P], F32, tag="ps_w")
                    for i in range(4):
                        mc = mg * 4 + i
                        nc.tensor.transpose(tr[:, i * P:(i + 1) * P],
                                            attn[1][:, mc * P:(mc + 1) * P], ident[:])
                    if mg == 0:
                        nc.scalar.copy(aT[:, 0:512], tr[:])
                    else:
                        nc.vector.tensor_copy(aT[:, 512:1024], tr[:])

                # out = attn_sel @ y
                o_ps = ps_o.tile([P, D], F32, tag="out")
                for mc in range(8):
                    nc.tensor.matmul(o_ps[:], aT[:, mc * P:(mc + 1) * P],
                                     y_nat[:, mc, :],
                                     start=(mc == 0), stop=(mc == 7))
                o_sb = sb.tile([P, D], F32, tag="osb")
                nc.vector.tensor_copy(o_sb[:], o_ps[:])
                nc.sync.dma_start(out=out_d[b], in_=o_sb[:])


def kernel(x, y, coords, U, S1, S2, gating, h_temp, p_temp, pos_emb):
    x = np.ascontiguousarray(np.asarray(x, dtype=np.float32))
    y = np.ascontiguousarray(np.asarray(y, dtype=np.float32))
    coords = np.ascontiguousarray(np.asarray(coords, dtype=np.float32))
    U = np.ascontiguousarray(np.asarray(U, dtype=np.float32))
    s_np = np.ascontiguousarray(
        np.concatenate([np.asarray(S1, dtype=np.float32),
                        np.asarray(S2, dtype=np.float32)], axis=1))
    pe_np = np.ascontiguousarray(np.asarray(pos_emb, dtype=np.float32)[:, :, 0])

    g = float(1.0 / (1.0 + np.exp(-float(gating))))
    h = float(h_temp)
    p = float(p_temp)

    key = (g, h, p)
    if key not in _CACHE:
        _CACHE[key] = _build(g, h, p)
    nc = _CACHE[key]

    in_maps = []
    for i in range(NCORES):
        qs = slice(i * P, (i + 1) * P)
        in_maps.append({
            "x": np.ascontiguousarray(x[:, qs, :]),
            "y": y,
            "coords": np.ascontiguousarray(coords[qs]),
            "pe": np.ascontiguousarray(pe_np[qs]),
            "u": U,
            "s": s_np,
        })

    res = run_bass_kernel_spmd(nc, in_maps, list(range(NCORES)))
    out = np.concatenate([r["out"] for r in res.results], axis=1)
    heat = np.concatenate([r["heat"] for r in res.results], axis=1)
    return out, heat


if __name__ == "__main__":
    rng = np.random.default_rng(0)
    inputs = {
        "x": rng.standard_normal((B, N, D), dtype=np.float32),
        "y": rng.standard_normal((B, N, D), dtype=np.float32),
        "coords": rng.standard_normal((N, N, 6), dtype=np.float32),
        "U": rng.standard_normal((D, D), dtype=np.float32) / np.sqrt(D),
        "S1": rng.standard_normal((D, 1), dtype=np.float32),
        "S2": rng.standard_normal((D, 1), dtype=np.float32),
        "gating": np.float32(1.0),
        "h_temp": np.float32(0.1),
        "p_temp": np.float32(1.0),
        "pos_emb": rng.random((N, 6, 1), dtype=np.float32),
    }
    out, heat = kernel(**inputs)
    print("out", out.shape, "heat", heat.shape)
